# revision 25
# baseline (speedup 1.0000x reference)
"""Bahdanau-attention Trainium2 kernel (data-parallel over 8 NeuronCores).

Computation (per batch row b):
    energy[s, d] = tanh(hidden[b] @ W_h + enc[b, s] @ W_e + b_attn)   [S, D]
    scores[s]    = energy[s] . w_v                                     [S]
    attn         = softmax(scores)                                     [S]
    out[b]       = sum_s attn[s] * enc[b, s]                           [E]

Device mapping (per core, 8 batches):
  - enc is staged host-side as encT: [b, e, s] in bf16 (e on SBUF partitions)
    so the big matmul streams through the PE with W_e chunks stationary,
    producing energy in [d, s] layout (d on partitions).
  - tanh + bias fused in one ScalarE activation per (b, d-chunk) with the
    per-partition bias column h_projT[:, b] + b_attn.
  - scores: DVE per-partition multiply by w_v, pairwise chunk adds, then a
    PE ones-matvec to reduce the 128 partitions.
  - softmax batched over the group's rows [G, S]; exp + row-sum fused via
    activation(accum_out=...).
  - weighted sum: attn row broadcast to 128 partitions via a stride-0 DMA,
    then one DVE tensor_tensor_reduce per (b, e-chunk) straight from the
    resident encT tiles (single pass over enc from HBM).
"""

import os
import numpy as np

B, S, ENC, DEC = 64, 2048, 512, 512
NCORES = 8
BL = B // NCORES          # batches per core
P = 128
EC = ENC // P             # 4 e-chunks
DC = DEC // P             # 4 d-chunks
KC = DEC // P             # 4 k-chunks (hidden dim)
GRP = 2                   # batches per softmax/wsum group
ST = 512                  # matmul moving free-dim tile
HT = 1024                 # psum energy tile free size
N_RED_DVE = 3             # of every 8 wsum reduces, how many run on DVE

_PROGRAM = None


def _build_program():
    import concourse.mybir as mybir
    import concourse.tile as tile
    from concourse import bacc
    from concourse.masks import make_identity
    from contextlib import ExitStack

    fp32 = mybir.dt.float32
    bf16 = mybir.dt.bfloat16
    AF = mybir.ActivationFunctionType
    ALU = mybir.AluOpType

    nc = bacc.Bacc("TRN2", debug=False, target_bir_lowering=False,
                   num_devices=NCORES)

    enc_d = nc.dram_tensor("encT", [BL, EC, P, S], bf16, kind="ExternalInput").ap()
    hid_d = nc.dram_tensor("hiddenT", [KC, P, BL], fp32, kind="ExternalInput").ap()
    wh_d = nc.dram_tensor("whT", [KC, P, DEC], fp32, kind="ExternalInput").ap()
    we_d = nc.dram_tensor("weT", [EC, P, DEC], bf16, kind="ExternalInput").ap()
    battn_d = nc.dram_tensor("battn", [P, DC], fp32, kind="ExternalInput").ap()
    wv_d = nc.dram_tensor("wv", [P, DC], fp32, kind="ExternalInput").ap()
    out_d = nc.dram_tensor("out", [BL, ENC], fp32, kind="ExternalOutput").ap()
    attn_d = nc.dram_tensor("attn_scratch", [BL, S], bf16).ap()

    with tile.TileContext(nc) as tc, ExitStack() as ctx:
        const = ctx.enter_context(tc.tile_pool(name="const", bufs=1))
        ps_e = ctx.enter_context(tc.tile_pool(name="ps_e", bufs=3, space="PSUM"))
        ps_sc = ctx.enter_context(tc.tile_pool(name="ps_sc", bufs=2, space="PSUM"))
        enc_pool = ctx.enter_context(tc.tile_pool(name="encp", bufs=22))
        tanh_pool = ctx.enter_context(tc.tile_pool(name="tanhp", bufs=4))
        wve_pool = ctx.enter_context(tc.tile_pool(name="wvep", bufs=6))
        wvs_pool = ctx.enter_context(tc.tile_pool(name="wvsp", bufs=4))
        arep_pool = ctx.enter_context(tc.tile_pool(name="arepp", bufs=3))
        junk_pool = ctx.enter_context(tc.tile_pool(name="junkp", bufs=3))
        junk2_pool = ctx.enter_context(tc.tile_pool(name="junk2p", bufs=2))
        smax_pool = ctx.enter_context(tc.tile_pool(name="smaxp", bufs=1))
        stage_pool = ctx.enter_context(tc.tile_pool(name="stagep", bufs=4))

        we_sb = const.tile([P, EC, DEC], bf16)
        wh_sb = const.tile([P, KC, DEC], fp32)
        hid_sb = const.tile([P, KC, BL], fp32)
        battn_sb = const.tile([P, DC], fp32)
        wv_sb = const.tile([P, DC], fp32)
        ones_sb = const.tile([P, 1], bf16)
        ident_sb = const.tile([P, P], fp32)
        biasT_sb = const.tile([P, DC, BL], fp32)
        outT_sb = const.tile([P, EC, BL], fp32)
        out_sb = const.tile([BL, ENC], fp32)

        nc.sync.dma_start(we_sb[:], we_d.rearrange("c p d -> p c d"))
        nc.sync.dma_start(wh_sb[:], wh_d.rearrange("c p d -> p c d"))
        nc.sync.dma_start(hid_sb[:], hid_d.rearrange("c p b -> p c b"))
        nc.sync.dma_start(battn_sb[:], battn_d)
        nc.sync.dma_start(wv_sb[:], wv_d)
        nc.vector.memset(ones_sb[:], 1.0)
        make_identity(nc, ident_sb[:])

        # h_projT[d, b] = sum_k W_h[k, d] * hidden[b, k]; biasT = h_projT + b_attn
        for dc in range(DC):
            hp_ps = ps_e.tile([P, BL], fp32, tag="pse")
            for kc in range(KC):
                nc.tensor.matmul(
                    hp_ps[:],
                    lhsT=wh_sb[:, kc, dc * P:(dc + 1) * P],
                    rhs=hid_sb[:, kc, :],
                    start=(kc == 0), stop=(kc == KC - 1))
            nc.scalar.activation(biasT_sb[:, dc, :], hp_ps[:], AF.Identity,
                                 bias=battn_sb[:, dc:dc + 1])

        enc_t = {}
        n_red = 0

        def wsum_batch(b):
            """Weighted sum for batch b; emitted one group after b's attn is
            computed so every dependency is ready when these ops reach the
            engine queue heads (avoids ACT/DVE head-of-line stalls)."""
            nonlocal n_red
            arep = arep_pool.tile([P, S], bf16, tag="arep", name=f"arep{b}")
            nc.sync.dma_start(arep[:], attn_d[b:b + 1, :].to_broadcast((P, S)))
            for ec in range(EC):
                prod = junk_pool.tile([P, S], bf16, tag="junk",
                                      name=f"prod{b}_{ec}")
                nc.vector.tensor_mul(prod[:], enc_t[(b, ec)][:], arep[:])
                if (n_red % 8) < N_RED_DVE:
                    nc.vector.tensor_reduce(
                        outT_sb[:, ec, b:b + 1], prod[:],
                        axis=mybir.AxisListType.X, op=ALU.add)
                else:
                    jk = junk2_pool.tile([P, S], bf16, tag="junk2",
                                         name=f"jk{b}_{ec}")
                    nc.scalar.activation(jk[:], prod[:], AF.Identity,
                                         accum_out=outT_sb[:, ec, b:b + 1])
                n_red += 1
                enc_t.pop((b, ec))

        NG = BL // GRP
        for g in range(NG):
            bs = list(range(g * GRP, (g + 1) * GRP))
            for b in bs:
                for ec in range(EC):
                    t = enc_pool.tile([P, S], bf16, tag="enc",
                                      name=f"enc{b}_{ec}")
                    nc.sync.dma_start(t[:], enc_d[b, ec])
                    enc_t[(b, ec)] = t

            scores_g = smax_pool.tile([GRP, S], fp32, tag="scores",
                                      name=f"scores{g}", bufs=2)

            for j, b in enumerate(bs):
                for h in range(S // HT):
                    wve = {}
                    for dc in range(DC):
                        eps = ps_e.tile([P, HT], fp32, tag="pse",
                                        name=f"eps{b}_{h}_{dc}")
                        for st in range(HT // ST):
                            for ec in range(EC):
                                nc.tensor.matmul(
                                    eps[:, st * ST:(st + 1) * ST],
                                    lhsT=we_sb[:, ec, dc * P:(dc + 1) * P],
                                    rhs=enc_t[(b, ec)][:, h * HT + st * ST:
                                                       h * HT + (st + 1) * ST],
                                    start=(ec == 0), stop=(ec == EC - 1))
                        tanh_t = tanh_pool.tile([P, HT], bf16, tag="tanh",
                                                name=f"tanh{b}_{h}_{dc}")
                        nc.scalar.activation(tanh_t[:], eps[:], AF.Tanh,
                                             bias=biasT_sb[:, dc, b:b + 1])
                        wve_t = wve_pool.tile([P, HT], bf16, tag="wve",
                                              name=f"wve{b}_{h}_{dc}")
                        nc.vector.tensor_scalar_mul(wve_t[:], tanh_t[:],
                                                    wv_sb[:, dc:dc + 1])
                        wve[dc] = wve_t

                    a01 = wvs_pool.tile([P, HT], bf16, tag="wvs",
                                        name=f"a01_{b}_{h}")
                    nc.gpsimd.tensor_add(a01[:], wve[0][:], wve[1][:])
                    a23 = wvs_pool.tile([P, HT], bf16, tag="wvs",
                                        name=f"a23_{b}_{h}")
                    nc.vector.tensor_add(a23[:], wve[2][:], wve[3][:])
                    asum = wvs_pool.tile([P, HT], bf16, tag="wvs",
                                         name=f"asum{b}_{h}")
                    nc.vector.tensor_add(asum[:], a01[:], a23[:])
                    for st in range(HT // ST):
                        sc_row = ps_sc.tile([1, ST], fp32, tag="sc",
                                            name=f"scr{b}_{h}_{st}")
                        nc.tensor.matmul(sc_row[:], lhsT=ones_sb[:],
                                         rhs=asum[:, st * ST:(st + 1) * ST],
                                         start=True, stop=True)
                        stg = stage_pool.tile([1, ST], fp32, tag="stg",
                                              name=f"stg{b}_{h}_{st}")
                        nc.scalar.copy(stg[:], sc_row[:])
                        s0 = h * HT + st * ST
                        nc.sync.dma_start(scores_g[j:j + 1, s0:s0 + ST], stg[:])

                # interleave the previous group's weighted sums between this
                # group's batches (their deps resolved a whole group ago)
                if g > 0:
                    wsum_batch((g - 1) * GRP + j)

            negmax_g = smax_pool.tile([GRP, 1], fp32, tag="negmax",
                                      name=f"negmax{g}")
            sumexp_g = smax_pool.tile([GRP, 1], fp32, tag="sumexp",
                                      name=f"sumexp{g}")
            rsum_g = smax_pool.tile([GRP, 1], fp32, tag="rsum",
                                    name=f"rsum{g}")
            probs_g = smax_pool.tile([GRP, S], fp32, tag="probs",
                                     name=f"probs{g}")
            attn_g = smax_pool.tile([GRP, S], bf16, tag="attn",
                                    name=f"attn{g}")
            nc.vector.tensor_reduce(negmax_g[:], scores_g[:],
                                    axis=mybir.AxisListType.X, op=ALU.max,
                                    negate=True)
            nc.scalar.activation(probs_g[:], scores_g[:], AF.Exp,
                                 bias=negmax_g[:], accum_out=sumexp_g[:])
            nc.vector.reciprocal(rsum_g[:], sumexp_g[:])
            nc.vector.tensor_scalar_mul(attn_g[:], probs_g[:], rsum_g[:])
            nc.sync.dma_start(attn_d[g * GRP:(g + 1) * GRP], attn_g[:])

        for b in range((NG - 1) * GRP, NG * GRP):
            wsum_batch(b)

        for ec in range(EC):
            tp = ps_e.tile([BL, P], fp32, tag="pse")
            nc.tensor.transpose(tp[:], outT_sb[:, ec, :], ident_sb[:])
            nc.scalar.copy(out_sb[:, ec * P:(ec + 1) * P], tp[:])
        nc.sync.dma_start(out_d[:], out_sb[:])

    nc.compile()
    return nc


def _get_program():
    global _PROGRAM
    if _PROGRAM is None:
        _PROGRAM = _build_program()
    return _PROGRAM


def _make_in_maps(hidden, encoder_outputs, W_attn, b_attn, w_v):
    import ml_dtypes
    bf = ml_dtypes.bfloat16
    W_h, W_e = W_attn[:DEC], W_attn[DEC:]
    whT = np.ascontiguousarray(np.asarray(W_h, np.float32).reshape(KC, P, DEC))
    weT = np.ascontiguousarray(np.asarray(W_e).reshape(EC, P, DEC).astype(bf))
    battn = np.ascontiguousarray(np.asarray(b_attn, np.float32).reshape(DC, P).T)
    wv = np.ascontiguousarray(np.asarray(w_v, np.float32).reshape(DC, P).T)
    in_maps = []
    for c in range(NCORES):
        hb = np.asarray(hidden[c * BL:(c + 1) * BL], np.float32)
        eb = np.asarray(encoder_outputs[c * BL:(c + 1) * BL])
        hidT = np.ascontiguousarray(hb.T.reshape(KC, P, BL))
        encT = np.ascontiguousarray(
            eb.transpose(0, 2, 1).reshape(BL, EC, P, S).astype(bf))
        in_maps.append({"encT": encT, "hiddenT": hidT, "whT": whT, "weT": weT,
                        "battn": battn, "wv": wv})
    return in_maps


def _install_trace_hooks():
    """The agent image's antenv lacks axon_hooks; recreate it from the
    ctypes NTFF profile shim in trn_agent_boot, and stub the fish-bucket
    artifact upload so the trace path stays local."""
    import sys, types
    if "antenv.axon_hooks" not in sys.modules:
        mod = types.ModuleType("antenv.axon_hooks")
        mod._hook = None
        mod.set_axon_ntff_profile_hook = lambda h: setattr(mod, "_hook", h)
        mod.get_axon_ntff_profile_hook = lambda: mod._hook
        sys.modules["antenv.axon_hooks"] = mod
        import antenv
        antenv.axon_hooks = mod
        try:
            from trn_agent_boot.trn_boot import _ntff_profile_via_ctypes
            mod._hook = _ntff_profile_via_ctypes("/opt/axon/libaxon_pjrt.so")
        except Exception as e:
            print(f"NTFF hook install failed: {e}")
    import concourse.bass_utils as bu
    bu.upload_artifacts = lambda tmpdir: f"local:{tmpdir}"


def run(hidden, encoder_outputs, W_attn, b_attn, w_v, trace=False, tmpdir=None):
    from concourse.bass_utils import run_bass_kernel_spmd
    if trace:
        _install_trace_hooks()
    nc = _get_program()
    in_maps = _make_in_maps(hidden, encoder_outputs, W_attn, b_attn, w_v)
    res = run_bass_kernel_spmd(nc, in_maps, list(range(NCORES)),
                               trace=trace, tmpdir=tmpdir)
    out = np.concatenate([np.asarray(res.results[c]["out"], np.float32)
                          for c in range(NCORES)], axis=0)
    return out, res


def kernel(hidden, encoder_outputs, W_attn, b_attn, w_v):
    out, _ = run(hidden, encoder_outputs, W_attn, b_attn, w_v)
    return out


# revision 26
# speedup vs baseline: 1.2225x; 1.2225x over previous
"""Bahdanau-attention Trainium2 kernel (data-parallel over 8 NeuronCores).

Computation (per batch row b):
    energy[s, d] = tanh(hidden[b] @ W_h + enc[b, s] @ W_e + b_attn)   [S, D]
    scores[s]    = energy[s] . w_v                                     [S]
    attn         = softmax(scores)                                     [S]
    out[b]       = sum_s attn[s] * enc[b, s]                           [E]

Device mapping (per core, 8 batches):
  - enc is staged host-side as encT: [b, e, s] in bf16 (e on SBUF partitions)
    so the big matmul streams through the PE with W_e chunks stationary,
    producing energy in [d, s] layout (d on partitions).
  - tanh + bias fused in one ScalarE activation per (b, d-chunk) with the
    per-partition bias column h_projT[:, b] + b_attn.
  - scores: DVE per-partition multiply by w_v, pairwise chunk adds, then a
    PE ones-matvec to reduce the 128 partitions.
  - softmax batched over the group's rows [G, S]; exp + row-sum fused via
    activation(accum_out=...).
  - weighted sum: attn row broadcast to 128 partitions via a stride-0 DMA,
    then one DVE tensor_tensor_reduce per (b, e-chunk) straight from the
    resident encT tiles (single pass over enc from HBM).
"""

import os
import numpy as np

B, S, ENC, DEC = 64, 2048, 512, 512
NCORES = 8
BL = B // NCORES          # batches per core
P = 128
EC = ENC // P             # 4 e-chunks
DC = DEC // P             # 4 d-chunks
KC = DEC // P             # 4 k-chunks (hidden dim)
GRP = 2                   # batches per softmax/wsum group
ST = 512                  # matmul moving free-dim tile
HT = 1024                 # psum energy tile free size
N_RED_DVE = 3             # of every 8 wsum reduces, how many run on DVE

_PROGRAM = None


def _build_program():
    import concourse.mybir as mybir
    import concourse.tile as tile
    from concourse import bacc
    from concourse.masks import make_identity
    from contextlib import ExitStack

    fp32 = mybir.dt.float32
    bf16 = mybir.dt.bfloat16
    AF = mybir.ActivationFunctionType
    ALU = mybir.AluOpType

    nc = bacc.Bacc("TRN2", debug=False, target_bir_lowering=False,
                   num_devices=NCORES)

    enc_d = nc.dram_tensor("encT", [BL, EC, P, S], bf16, kind="ExternalInput").ap()
    hid_d = nc.dram_tensor("hiddenT", [KC, P, BL], fp32, kind="ExternalInput").ap()
    wh_d = nc.dram_tensor("whT", [KC, P, DEC], fp32, kind="ExternalInput").ap()
    we_d = nc.dram_tensor("weT", [EC, P, DEC], bf16, kind="ExternalInput").ap()
    battn_d = nc.dram_tensor("battn", [P, DC], fp32, kind="ExternalInput").ap()
    wv_d = nc.dram_tensor("wv", [P, DC], fp32, kind="ExternalInput").ap()
    out_d = nc.dram_tensor("out", [BL, ENC], fp32, kind="ExternalOutput").ap()
    attn_d = nc.dram_tensor("attn_scratch", [BL, S], bf16).ap()

    with tile.TileContext(nc) as tc, ExitStack() as ctx:
        const = ctx.enter_context(tc.tile_pool(name="const", bufs=1))
        ps_e = ctx.enter_context(tc.tile_pool(name="ps_e", bufs=3, space="PSUM"))
        ps_sc = ctx.enter_context(tc.tile_pool(name="ps_sc", bufs=2, space="PSUM"))
        enc_pool = ctx.enter_context(tc.tile_pool(name="encp", bufs=22))
        tanh_pool = ctx.enter_context(tc.tile_pool(name="tanhp", bufs=4))
        wve_pool = ctx.enter_context(tc.tile_pool(name="wvep", bufs=6))
        wvs_pool = ctx.enter_context(tc.tile_pool(name="wvsp", bufs=4))
        arep_pool = ctx.enter_context(tc.tile_pool(name="arepp", bufs=3))
        junk_pool = ctx.enter_context(tc.tile_pool(name="junkp", bufs=3))
        junk2_pool = ctx.enter_context(tc.tile_pool(name="junk2p", bufs=2))
        smax_pool = ctx.enter_context(tc.tile_pool(name="smaxp", bufs=1))
        stage_pool = ctx.enter_context(tc.tile_pool(name="stagep", bufs=4))

        we_sb = const.tile([P, EC, DEC], bf16)
        wh_sb = const.tile([P, KC, DEC], fp32)
        hid_sb = const.tile([P, KC, BL], fp32)
        battn_sb = const.tile([P, DC], fp32)
        wv_sb = const.tile([P, DC], fp32)
        ones_sb = const.tile([P, 1], bf16)
        ident_sb = const.tile([P, P], fp32)
        biasT_sb = const.tile([P, DC, BL], fp32)
        outT_sb = const.tile([P, EC, BL], fp32)
        out_sb = const.tile([BL, ENC], fp32)

        nc.sync.dma_start(we_sb[:], we_d.rearrange("c p d -> p c d"))
        nc.sync.dma_start(wh_sb[:], wh_d.rearrange("c p d -> p c d"))
        nc.sync.dma_start(hid_sb[:], hid_d.rearrange("c p b -> p c b"))
        nc.sync.dma_start(battn_sb[:], battn_d)
        nc.sync.dma_start(wv_sb[:], wv_d)
        nc.vector.memset(ones_sb[:], 1.0)
        make_identity(nc, ident_sb[:])

        # h_projT[d, b] = sum_k W_h[k, d] * hidden[b, k]; biasT = h_projT + b_attn
        for dc in range(DC):
            hp_ps = ps_e.tile([P, BL], fp32, tag="pse")
            for kc in range(KC):
                nc.tensor.matmul(
                    hp_ps[:],
                    lhsT=wh_sb[:, kc, dc * P:(dc + 1) * P],
                    rhs=hid_sb[:, kc, :],
                    start=(kc == 0), stop=(kc == KC - 1))
            nc.scalar.activation(biasT_sb[:, dc, :], hp_ps[:], AF.Identity,
                                 bias=battn_sb[:, dc:dc + 1])

        enc_t = {}
        n_red = 0

        def wsum_batch(b):
            """Weighted sum for batch b; emitted one group after b's attn is
            computed so every dependency is ready when these ops reach the
            engine queue heads (avoids ACT/DVE head-of-line stalls)."""
            nonlocal n_red
            arep = arep_pool.tile([P, S], bf16, tag="arep", name=f"arep{b}")
            nc.sync.dma_start(arep[:], attn_d[b:b + 1, :].to_broadcast((P, S)))
            for ec in range(EC):
                prod = junk_pool.tile([P, S], bf16, tag="junk",
                                      name=f"prod{b}_{ec}")
                nc.vector.tensor_mul(prod[:], enc_t[(b, ec)][:], arep[:])
                if (n_red % 8) < N_RED_DVE:
                    nc.vector.tensor_reduce(
                        outT_sb[:, ec, b:b + 1], prod[:],
                        axis=mybir.AxisListType.X, op=ALU.add)
                else:
                    jk = junk2_pool.tile([P, S], bf16, tag="junk2",
                                         name=f"jk{b}_{ec}")
                    nc.scalar.activation(jk[:], prod[:], AF.Identity,
                                         accum_out=outT_sb[:, ec, b:b + 1])
                n_red += 1
                enc_t.pop((b, ec))

        NG = BL // GRP
        for g in range(NG):
            bs = list(range(g * GRP, (g + 1) * GRP))
            for b in bs:
                for ec in range(EC):
                    t = enc_pool.tile([P, S], bf16, tag="enc",
                                      name=f"enc{b}_{ec}")
                    nc.sync.dma_start(t[:], enc_d[b, ec])
                    enc_t[(b, ec)] = t

            scores_g = smax_pool.tile([GRP, S], fp32, tag="scores",
                                      name=f"scores{g}", bufs=2)

            for j, b in enumerate(bs):
                for h in range(S // HT):
                    wve = {}
                    for dc in range(DC):
                        eps = ps_e.tile([P, HT], fp32, tag="pse",
                                        name=f"eps{b}_{h}_{dc}")
                        for st in range(HT // ST):
                            for ec in range(EC):
                                nc.tensor.matmul(
                                    eps[:, st * ST:(st + 1) * ST],
                                    lhsT=we_sb[:, ec, dc * P:(dc + 1) * P],
                                    rhs=enc_t[(b, ec)][:, h * HT + st * ST:
                                                       h * HT + (st + 1) * ST],
                                    start=(ec == 0), stop=(ec == EC - 1))
                        tanh_t = tanh_pool.tile([P, HT], bf16, tag="tanh",
                                                name=f"tanh{b}_{h}_{dc}")
                        nc.scalar.activation(tanh_t[:], eps[:], AF.Tanh,
                                             bias=biasT_sb[:, dc, b:b + 1])
                        wve_t = wve_pool.tile([P, HT], bf16, tag="wve",
                                              name=f"wve{b}_{h}_{dc}")
                        nc.vector.tensor_scalar_mul(wve_t[:], tanh_t[:],
                                                    wv_sb[:, dc:dc + 1])
                        wve[dc] = wve_t

                    a01 = wvs_pool.tile([P, HT], bf16, tag="wvs",
                                        name=f"a01_{b}_{h}")
                    nc.vector.tensor_add(a01[:], wve[0][:], wve[1][:])
                    a23 = wvs_pool.tile([P, HT], bf16, tag="wvs",
                                        name=f"a23_{b}_{h}")
                    nc.vector.tensor_add(a23[:], wve[2][:], wve[3][:])
                    asum = wvs_pool.tile([P, HT], bf16, tag="wvs",
                                         name=f"asum{b}_{h}")
                    nc.vector.tensor_add(asum[:], a01[:], a23[:])
                    for st in range(HT // ST):
                        sc_row = ps_sc.tile([1, ST], fp32, tag="sc",
                                            name=f"scr{b}_{h}_{st}")
                        nc.tensor.matmul(sc_row[:], lhsT=ones_sb[:],
                                         rhs=asum[:, st * ST:(st + 1) * ST],
                                         start=True, stop=True)
                        stg = stage_pool.tile([1, ST], fp32, tag="stg",
                                              name=f"stg{b}_{h}_{st}")
                        nc.scalar.copy(stg[:], sc_row[:])
                        s0 = h * HT + st * ST
                        nc.sync.dma_start(scores_g[j:j + 1, s0:s0 + ST], stg[:])

                # interleave the previous group's weighted sums between this
                # group's batches (their deps resolved a whole group ago)
                if g > 0:
                    wsum_batch((g - 1) * GRP + j)

            negmax_g = smax_pool.tile([GRP, 1], fp32, tag="negmax",
                                      name=f"negmax{g}")
            sumexp_g = smax_pool.tile([GRP, 1], fp32, tag="sumexp",
                                      name=f"sumexp{g}")
            rsum_g = smax_pool.tile([GRP, 1], fp32, tag="rsum",
                                    name=f"rsum{g}")
            probs_g = smax_pool.tile([GRP, S], fp32, tag="probs",
                                     name=f"probs{g}")
            attn_g = smax_pool.tile([GRP, S], bf16, tag="attn",
                                    name=f"attn{g}")
            nc.vector.tensor_reduce(negmax_g[:], scores_g[:],
                                    axis=mybir.AxisListType.X, op=ALU.max,
                                    negate=True)
            nc.scalar.activation(probs_g[:], scores_g[:], AF.Exp,
                                 bias=negmax_g[:], accum_out=sumexp_g[:])
            nc.vector.reciprocal(rsum_g[:], sumexp_g[:])
            nc.vector.tensor_scalar_mul(attn_g[:], probs_g[:], rsum_g[:])
            nc.sync.dma_start(attn_d[g * GRP:(g + 1) * GRP], attn_g[:])

        for b in range((NG - 1) * GRP, NG * GRP):
            wsum_batch(b)

        for ec in range(EC):
            tp = ps_e.tile([BL, P], fp32, tag="pse")
            nc.tensor.transpose(tp[:], outT_sb[:, ec, :], ident_sb[:])
            nc.scalar.copy(out_sb[:, ec * P:(ec + 1) * P], tp[:])
        nc.sync.dma_start(out_d[:], out_sb[:])

    nc.compile()
    return nc


def _get_program():
    global _PROGRAM
    if _PROGRAM is None:
        _PROGRAM = _build_program()
    return _PROGRAM


def _make_in_maps(hidden, encoder_outputs, W_attn, b_attn, w_v):
    import ml_dtypes
    bf = ml_dtypes.bfloat16
    W_h, W_e = W_attn[:DEC], W_attn[DEC:]
    whT = np.ascontiguousarray(np.asarray(W_h, np.float32).reshape(KC, P, DEC))
    weT = np.ascontiguousarray(np.asarray(W_e).reshape(EC, P, DEC).astype(bf))
    battn = np.ascontiguousarray(np.asarray(b_attn, np.float32).reshape(DC, P).T)
    wv = np.ascontiguousarray(np.asarray(w_v, np.float32).reshape(DC, P).T)
    in_maps = []
    for c in range(NCORES):
        hb = np.asarray(hidden[c * BL:(c + 1) * BL], np.float32)
        eb = np.asarray(encoder_outputs[c * BL:(c + 1) * BL])
        hidT = np.ascontiguousarray(hb.T.reshape(KC, P, BL))
        encT = np.ascontiguousarray(
            eb.transpose(0, 2, 1).reshape(BL, EC, P, S).astype(bf))
        in_maps.append({"encT": encT, "hiddenT": hidT, "whT": whT, "weT": weT,
                        "battn": battn, "wv": wv})
    return in_maps


def _install_trace_hooks():
    """The agent image's antenv lacks axon_hooks; recreate it from the
    ctypes NTFF profile shim in trn_agent_boot, and stub the fish-bucket
    artifact upload so the trace path stays local."""
    import sys, types
    if "antenv.axon_hooks" not in sys.modules:
        mod = types.ModuleType("antenv.axon_hooks")
        mod._hook = None
        mod.set_axon_ntff_profile_hook = lambda h: setattr(mod, "_hook", h)
        mod.get_axon_ntff_profile_hook = lambda: mod._hook
        sys.modules["antenv.axon_hooks"] = mod
        import antenv
        antenv.axon_hooks = mod
        try:
            from trn_agent_boot.trn_boot import _ntff_profile_via_ctypes
            mod._hook = _ntff_profile_via_ctypes("/opt/axon/libaxon_pjrt.so")
        except Exception as e:
            print(f"NTFF hook install failed: {e}")
    import concourse.bass_utils as bu
    bu.upload_artifacts = lambda tmpdir: f"local:{tmpdir}"


def run(hidden, encoder_outputs, W_attn, b_attn, w_v, trace=False, tmpdir=None):
    from concourse.bass_utils import run_bass_kernel_spmd
    if trace:
        _install_trace_hooks()
    nc = _get_program()
    in_maps = _make_in_maps(hidden, encoder_outputs, W_attn, b_attn, w_v)
    res = run_bass_kernel_spmd(nc, in_maps, list(range(NCORES)),
                               trace=trace, tmpdir=tmpdir)
    out = np.concatenate([np.asarray(res.results[c]["out"], np.float32)
                          for c in range(NCORES)], axis=0)
    return out, res


def kernel(hidden, encoder_outputs, W_attn, b_attn, w_v):
    out, _ = run(hidden, encoder_outputs, W_attn, b_attn, w_v)
    return out


# revision 38
# speedup vs baseline: 1.2701x; 1.0390x over previous
"""Bahdanau-attention Trainium2 kernel (data-parallel over 8 NeuronCores).

Computation (per batch row b):
    energy[s, d] = tanh(hidden[b] @ W_h + enc[b, s] @ W_e + b_attn)   [S, D]
    scores[s]    = energy[s] . w_v                                     [S]
    attn         = softmax(scores)                                     [S]
    out[b]       = sum_s attn[s] * enc[b, s]                           [E]

Device mapping (per core, 8 batches):
  - enc is staged host-side as encT: [b, e, s] in bf16 (e on SBUF partitions)
    so the big matmul streams through the PE with W_e chunks stationary,
    producing energy in [d, s] layout (d on partitions).
  - tanh + bias fused in one ScalarE activation per (b, d-chunk) with the
    per-partition bias column h_projT[:, b] + b_attn.
  - scores: DVE per-partition multiply by w_v, pairwise chunk adds, then a
    PE ones-matvec to reduce the 128 partitions.
  - softmax batched over the group's rows [G, S]; exp + row-sum fused via
    activation(accum_out=...).
  - weighted sum: attn row broadcast to 128 partitions via a stride-0 DMA,
    then one DVE tensor_tensor_reduce per (b, e-chunk) straight from the
    resident encT tiles (single pass over enc from HBM).
"""

import os
import numpy as np

B, S, ENC, DEC = 64, 2048, 512, 512
NCORES = 8
BL = B // NCORES          # batches per core
P = 128
EC = ENC // P             # 4 e-chunks
DC = DEC // P             # 4 d-chunks
KC = DEC // P             # 4 k-chunks (hidden dim)
GRP = 2                   # batches per softmax/wsum group
ST = 512                  # matmul moving free-dim tile
HT = 1024                 # psum energy tile free size
N_RED_DVE = 3             # of every 8 wsum reduces, how many run on DVE

_PROGRAM = None


def _build_program():
    import concourse.mybir as mybir
    import concourse.tile as tile
    from concourse import bacc
    from concourse.masks import make_identity
    from contextlib import ExitStack

    fp32 = mybir.dt.float32
    bf16 = mybir.dt.bfloat16
    AF = mybir.ActivationFunctionType
    ALU = mybir.AluOpType

    nc = bacc.Bacc("TRN2", debug=False, target_bir_lowering=False,
                   num_devices=NCORES)

    enc_d = nc.dram_tensor("encT", [BL, EC, P, S], bf16, kind="ExternalInput").ap()
    encn_d = nc.dram_tensor("encN", [BL, S, ENC], bf16, kind="ExternalInput").ap()
    hid_d = nc.dram_tensor("hiddenT", [KC, P, BL], fp32, kind="ExternalInput").ap()
    wh_d = nc.dram_tensor("whT", [KC, P, DEC], fp32, kind="ExternalInput").ap()
    we_d = nc.dram_tensor("weT", [EC, P, DEC], bf16, kind="ExternalInput").ap()
    battn_d = nc.dram_tensor("battn", [P, DC], fp32, kind="ExternalInput").ap()
    wv_d = nc.dram_tensor("wv", [P, DC], fp32, kind="ExternalInput").ap()
    out_d = nc.dram_tensor("out", [BL, ENC], fp32, kind="ExternalOutput").ap()
    attn_d = nc.dram_tensor("attn_scratch", [BL, S], bf16).ap()

    with tile.TileContext(nc) as tc, ExitStack() as ctx:
        const = ctx.enter_context(tc.tile_pool(name="const", bufs=1))
        ps_e = ctx.enter_context(tc.tile_pool(name="ps_e", bufs=3, space="PSUM"))
        ps_sc = ctx.enter_context(tc.tile_pool(name="ps_sc", bufs=2, space="PSUM"))
        enc_pool = ctx.enter_context(tc.tile_pool(name="encp", bufs=18))
        tanh_pool = ctx.enter_context(tc.tile_pool(name="tanhp", bufs=4))
        wve_pool = ctx.enter_context(tc.tile_pool(name="wvep", bufs=6))
        wvs_pool = ctx.enter_context(tc.tile_pool(name="wvsp", bufs=4))
        arep_pool = ctx.enter_context(tc.tile_pool(name="arepp", bufs=2))
        junk_pool = ctx.enter_context(tc.tile_pool(name="junkp", bufs=2))
        junk2_pool = ctx.enter_context(tc.tile_pool(name="junk2p", bufs=2))
        smax_pool = ctx.enter_context(tc.tile_pool(name="smaxp", bufs=1))
        stage_pool = ctx.enter_context(tc.tile_pool(name="stagep", bufs=4))
        encn_pool = ctx.enter_context(tc.tile_pool(name="encnp", bufs=12))

        we_sb = const.tile([P, EC, DEC], bf16)
        wh_sb = const.tile([P, KC, DEC], fp32)
        hid_sb = const.tile([P, KC, BL], fp32)
        battn_sb = const.tile([P, DC], fp32)
        wv_sb = const.tile([P, DC], fp32)
        ones_sb = const.tile([P, 1], bf16)
        ident_sb = const.tile([P, P], fp32)
        identb_sb = const.tile([P, P], bf16)
        attnT_sb = const.tile([P, S // P, GRP], bf16)
        biasT_sb = const.tile([P, DC, BL], fp32)
        outT_sb = const.tile([P, EC, BL], fp32)
        out_sb = const.tile([BL, ENC], fp32)

        nc.sync.dma_start(we_sb[:], we_d.rearrange("c p d -> p c d"))
        nc.sync.dma_start(wh_sb[:], wh_d.rearrange("c p d -> p c d"))
        nc.sync.dma_start(hid_sb[:], hid_d.rearrange("c p b -> p c b"))
        nc.sync.dma_start(battn_sb[:], battn_d)
        nc.sync.dma_start(wv_sb[:], wv_d)
        nc.vector.memset(ones_sb[:], 1.0)
        make_identity(nc, ident_sb[:])
        make_identity(nc, identb_sb[:])

        # h_projT[d, b] = sum_k W_h[k, d] * hidden[b, k]; biasT = h_projT + b_attn
        for dc in range(DC):
            hp_ps = ps_e.tile([P, BL], fp32, tag="pse")
            for kc in range(KC):
                nc.tensor.matmul(
                    hp_ps[:],
                    lhsT=wh_sb[:, kc, dc * P:(dc + 1) * P],
                    rhs=hid_sb[:, kc, :],
                    start=(kc == 0), stop=(kc == KC - 1))
            nc.scalar.activation(biasT_sb[:, dc, :], hp_ps[:], AF.Identity,
                                 bias=battn_sb[:, dc:dc + 1])

        enc_t = {}
        n_red = 0
        pending = []          # deferred wsum work units (closures)

        def wsum_units(b):
            """Weighted-sum work for batch b as fine-grained units, emitted a
            group after b's attn is computed so every dependency is ready when
            these ops reach the engine queue heads (no head-of-line stalls)."""
            def start():
                arep = arep_pool.tile([P, S], bf16, tag="arep", name=f"arep{b}")
                nc.sync.dma_start(arep[:],
                                  attn_d[b:b + 1, :].to_broadcast((P, S)))
                return arep
            state = {}

            def unit(ec):
                def run():
                    nonlocal n_red
                    if "arep" not in state:
                        state["arep"] = start()
                    arep = state["arep"]
                    prod = junk_pool.tile([P, S], bf16, tag="junk",
                                          name=f"prod{b}_{ec}")
                    nc.vector.tensor_mul(prod[:], enc_t[(b, ec)][:], arep[:])
                    if (n_red % 8) < N_RED_DVE:
                        nc.vector.tensor_reduce(
                            outT_sb[:, ec, b:b + 1], prod[:],
                            axis=mybir.AxisListType.X, op=ALU.add)
                    else:
                        jk = junk2_pool.tile([P, S], bf16, tag="junk2",
                                             name=f"jk{b}_{ec}")
                        nc.scalar.activation(jk[:], prod[:], AF.Identity,
                                             accum_out=outT_sb[:, ec, b:b + 1])
                    n_red += 1
                    enc_t.pop((b, ec))
                return run
            return [unit(ec) for ec in range(EC)]

        NG = BL // GRP
        for g in range(NG):
            bs = list(range(g * GRP, (g + 1) * GRP))
            for b in bs:
                for ec in range(EC):
                    t = enc_pool.tile([P, S], bf16, tag="enc",
                                      name=f"enc{b}_{ec}")
                    nc.sync.dma_start(t[:], enc_d[b, ec])
                    enc_t[(b, ec)] = t
            if g == NG - 1:
                # last group's weighted sum runs on the (otherwise idle) PE
                # at the kernel tail; stream natural-layout enc tiles for it
                encn_t = {}
                for b in bs:
                    for sc in range(S // P):
                        t = encn_pool.tile([P, ENC], bf16, tag="encn",
                                           name=f"encn{b}_{sc}")
                        nc.sync.dma_start(t[:], encn_d[b, sc * P:(sc + 1) * P, :])
                        encn_t[(b, sc)] = t

            scores_g = smax_pool.tile([GRP, S], fp32, tag="scores",
                                      name=f"scores{g}", bufs=2)

            for j, b in enumerate(bs):
                for h in range(S // HT):
                    wve = {}
                    for dc in range(DC):
                        eps = ps_e.tile([P, HT], fp32, tag="pse",
                                        name=f"eps{b}_{h}_{dc}")
                        for st in range(HT // ST):
                            for ec in range(EC):
                                nc.tensor.matmul(
                                    eps[:, st * ST:(st + 1) * ST],
                                    lhsT=we_sb[:, ec, dc * P:(dc + 1) * P],
                                    rhs=enc_t[(b, ec)][:, h * HT + st * ST:
                                                       h * HT + (st + 1) * ST],
                                    start=(ec == 0), stop=(ec == EC - 1))
                        tanh_t = tanh_pool.tile([P, HT], bf16, tag="tanh",
                                                name=f"tanh{b}_{h}_{dc}")
                        nc.scalar.activation(tanh_t[:], eps[:], AF.Tanh,
                                             bias=biasT_sb[:, dc, b:b + 1])
                        wve_t = wve_pool.tile([P, HT], bf16, tag="wve",
                                              name=f"wve{b}_{h}_{dc}")
                        nc.vector.tensor_scalar_mul(wve_t[:], tanh_t[:],
                                                    wv_sb[:, dc:dc + 1])
                        wve[dc] = wve_t

                    a01 = wvs_pool.tile([P, HT], bf16, tag="wvs",
                                        name=f"a01_{b}_{h}")
                    nc.vector.tensor_add(a01[:], wve[0][:], wve[1][:])
                    a23 = wvs_pool.tile([P, HT], bf16, tag="wvs",
                                        name=f"a23_{b}_{h}")
                    nc.vector.tensor_add(a23[:], wve[2][:], wve[3][:])
                    asum = wvs_pool.tile([P, HT], bf16, tag="wvs",
                                         name=f"asum{b}_{h}")
                    nc.vector.tensor_add(asum[:], a01[:], a23[:])
                    for st in range(HT // ST):
                        sc_row = ps_sc.tile([1, ST], fp32, tag="sc",
                                            name=f"scr{b}_{h}_{st}")
                        nc.tensor.matmul(sc_row[:], lhsT=ones_sb[:],
                                         rhs=asum[:, st * ST:(st + 1) * ST],
                                         start=True, stop=True)
                        stg = stage_pool.tile([1, ST], fp32, tag="stg",
                                              name=f"stg{b}_{h}_{st}")
                        nc.scalar.copy(stg[:], sc_row[:])
                        s0 = h * HT + st * ST
                        nc.sync.dma_start(scores_g[j:j + 1, s0:s0 + ST], stg[:])

                    # interleave the previous groups' deferred weighted-sum
                    # units between compute blocks (deps resolved a group ago)
                    for _ in range(2):
                        if pending:
                            pending.pop(0)()

            negmax_g = smax_pool.tile([GRP, 1], fp32, tag="negmax",
                                      name=f"negmax{g}")
            sumexp_g = smax_pool.tile([GRP, 1], fp32, tag="sumexp",
                                      name=f"sumexp{g}")
            rsum_g = smax_pool.tile([GRP, 1], fp32, tag="rsum",
                                    name=f"rsum{g}")
            probs_g = smax_pool.tile([GRP, S], bf16, tag="probs",
                                     name=f"probs{g}")
            attn_g = smax_pool.tile([GRP, S], bf16, tag="attn",
                                    name=f"attn{g}")
            nc.vector.tensor_reduce(negmax_g[:], scores_g[:],
                                    axis=mybir.AxisListType.X, op=ALU.max,
                                    negate=True)
            nc.scalar.activation(probs_g[:], scores_g[:], AF.Exp,
                                 bias=negmax_g[:], accum_out=sumexp_g[:])
            nc.vector.reciprocal(rsum_g[:], sumexp_g[:])
            nc.vector.tensor_scalar_mul(attn_g[:], probs_g[:], rsum_g[:])
            if g < NG - 1:
                nc.sync.dma_start(attn_d[g * GRP:(g + 1) * GRP], attn_g[:])
                for b in bs:
                    pending.extend(wsum_units(b))
            else:
                last_attn = attn_g

        # drain any remaining deferred DVE/ACT weighted-sum units
        while pending:
            pending.pop(0)()

        # tail: last group's weighted sum on PE — transpose attn rows to
        # columns, then attn-weighted matvecs over natural-layout enc tiles
        for sc in range(S // P):
            atp = ps_sc.tile([P, GRP], bf16, tag="sc", name=f"atp{sc}")
            nc.tensor.transpose(atp[:], last_attn[:, sc * P:(sc + 1) * P],
                                identb_sb[0:GRP, 0:GRP])
            nc.vector.tensor_copy(attnT_sb[:, sc, :], atp[:])
        for j in range(GRP):
            b = (NG - 1) * GRP + j
            orow = ps_e.tile([1, ENC], fp32, tag="pse", name=f"orow{b}")
            for sc in range(S // P):
                nc.tensor.matmul(orow[:], lhsT=attnT_sb[:, sc, j:j + 1],
                                 rhs=encn_t[(b, sc)][:],
                                 start=(sc == 0), stop=(sc == S // P - 1))
            ostg = stage_pool.tile([1, ENC], fp32, tag="stg", name=f"ostg{b}")
            nc.scalar.copy(ostg[:], orow[:])
            nc.sync.dma_start(out_d[b:b + 1, :], ostg[:])

        for ec in range(EC):
            tp = ps_e.tile([BL, P], fp32, tag="pse")
            nc.tensor.transpose(tp[:], outT_sb[:, ec, :], ident_sb[:])
            nc.scalar.copy(out_sb[:, ec * P:(ec + 1) * P], tp[:])
        nc.sync.dma_start(out_d[0:(NG - 1) * GRP], out_sb[0:(NG - 1) * GRP])

    nc.compile()
    return nc


def _get_program():
    global _PROGRAM
    if _PROGRAM is None:
        _PROGRAM = _build_program()
    return _PROGRAM


def _make_in_maps(hidden, encoder_outputs, W_attn, b_attn, w_v):
    import ml_dtypes
    bf = ml_dtypes.bfloat16
    W_h, W_e = W_attn[:DEC], W_attn[DEC:]
    whT = np.ascontiguousarray(np.asarray(W_h, np.float32).reshape(KC, P, DEC))
    weT = np.ascontiguousarray(np.asarray(W_e).reshape(EC, P, DEC).astype(bf))
    battn = np.ascontiguousarray(np.asarray(b_attn, np.float32).reshape(DC, P).T)
    wv = np.ascontiguousarray(np.asarray(w_v, np.float32).reshape(DC, P).T)
    in_maps = []
    for c in range(NCORES):
        hb = np.asarray(hidden[c * BL:(c + 1) * BL], np.float32)
        eb = np.asarray(encoder_outputs[c * BL:(c + 1) * BL])
        hidT = np.ascontiguousarray(hb.T.reshape(KC, P, BL))
        encT = np.ascontiguousarray(
            eb.transpose(0, 2, 1).reshape(BL, EC, P, S).astype(bf))
        encN = np.ascontiguousarray(eb.astype(bf))
        in_maps.append({"encT": encT, "encN": encN, "hiddenT": hidT,
                        "whT": whT, "weT": weT, "battn": battn, "wv": wv})
    return in_maps


def _install_trace_hooks():
    """The agent image's antenv lacks axon_hooks; recreate it from the
    ctypes NTFF profile shim in trn_agent_boot, and stub the fish-bucket
    artifact upload so the trace path stays local."""
    import sys, types
    if "antenv.axon_hooks" not in sys.modules:
        mod = types.ModuleType("antenv.axon_hooks")
        mod._hook = None
        mod.set_axon_ntff_profile_hook = lambda h: setattr(mod, "_hook", h)
        mod.get_axon_ntff_profile_hook = lambda: mod._hook
        sys.modules["antenv.axon_hooks"] = mod
        import antenv
        antenv.axon_hooks = mod
        try:
            from trn_agent_boot.trn_boot import _ntff_profile_via_ctypes
            mod._hook = _ntff_profile_via_ctypes("/opt/axon/libaxon_pjrt.so")
        except Exception as e:
            print(f"NTFF hook install failed: {e}")
    import concourse.bass_utils as bu
    bu.upload_artifacts = lambda tmpdir: f"local:{tmpdir}"


def run(hidden, encoder_outputs, W_attn, b_attn, w_v, trace=False, tmpdir=None):
    from concourse.bass_utils import run_bass_kernel_spmd
    if trace:
        _install_trace_hooks()
    nc = _get_program()
    in_maps = _make_in_maps(hidden, encoder_outputs, W_attn, b_attn, w_v)
    res = run_bass_kernel_spmd(nc, in_maps, list(range(NCORES)),
                               trace=trace, tmpdir=tmpdir)
    out = np.concatenate([np.asarray(res.results[c]["out"], np.float32)
                          for c in range(NCORES)], axis=0)
    return out, res


def kernel(hidden, encoder_outputs, W_attn, b_attn, w_v):
    out, _ = run(hidden, encoder_outputs, W_attn, b_attn, w_v)
    return out
